# revision 12
# baseline (speedup 1.0000x reference)
"""CropAndResize (TF-style, crop 14x14) on 8 Trainium2 NeuronCores.

Strategy (data-parallel over ROIs, balanced across cores):
  - Host: build a 2x2-patch image img4[b, h, w] = [tl, tr, bl, br] x 256ch
    in bf16 (channel-last), so every output pixel's four bilinear corners
    are ONE contiguous 2KB payload -> one gather descriptor per pixel.
  - Boxes are sorted by box_ind and split into 8 equal contiguous windows
    (125 boxes/core instead of max-per-image ~150). Each window touches at
    most 2 images, whose img4 tables are concatenated per core (int16
    descriptor indices still fit: 2*14949 < 32767). If an adversarial
    box_ind makes some window span >2 images, fall back to per-image
    grouping (correct, just slower).
  - Host computes the TF sampling grid (f32 mirror of the reference) and
    folds the bilinear lerp + validity mask into 4 corner weights per
    pixel, stored bf16 and replicated x8 so the device weight APs have a
    packed 2-byte last dim -> DVE runs every tensor_tensor at 2x rate.
  - Device: per chunk of 8 boxes (1664 padded pixels), SWDGE dma_gather
    (4 sub-gathers round-robined over 4 SWDGE queues to avoid ring-space
    stalls) fetches the 2KB patches (pixel on partitions, 4*256 channels
    on the free dim), then out = sum_k Wk*corner_k runs as 7 chunk-wide
    all-bf16 DVE tensor_tensor ops, and the bf16 result streams back to
    DRAM pixel-major. Host converts to f32 (well within tolerance).
"""

import numpy as np
import ml_dtypes

import concourse.bacc as bacc
import concourse.bass as bass
import concourse.tile as tile
from concourse import mybir, library_config, bass_utils

H, W, C = 100, 152, 256
HP, WP = H - 1, W - 1      # patch grid
NP4 = HP * WP              # 14949 gatherable 2x2 patches per image
EL = 4 * C                 # 1024 bf16 elements per patch payload (2KB)
CROP = 14
PX = CROP * CROP           # 196 pixels per box
P = 128                    # SBUF partitions
NCORES = 8
CH = 8                     # boxes per chunk
QPAD = ((CH * PX + P - 1) // P) * P   # padded pixels per chunk (1664)
S = QPAD // P              # output slots per chunk (13)
NI = QPAD                  # gather descriptors per chunk
R = 8                      # weight replication for packed bf16 APs
CB = C // R                # 32 channel blocks per corner

F32 = mybir.dt.float32
BF16 = mybir.dt.bfloat16
I16 = mybir.dt.int16
MULT = mybir.AluOpType.mult
ADD = mybir.AluOpType.add

_cache = {}
LAST_EXEC_NS = None


def _grid_params(boxes):
    """f32 mirror of the reference sampling-grid math."""
    f = np.float32
    y1, x1, y2, x2 = boxes[:, 0], boxes[:, 1], boxes[:, 2], boxes[:, 3]
    h_scale = (y2 - y1) * f(H - 1) / f(CROP - 1)
    w_scale = (x2 - x1) * f(W - 1) / f(CROP - 1)
    ar = np.arange(CROP, dtype=np.float32)
    in_y = y1[:, None] * f(H - 1) + ar[None, :] * h_scale[:, None]
    in_x = x1[:, None] * f(W - 1) + ar[None, :] * w_scale[:, None]
    valid_y = (in_y >= 0) & (in_y <= H - 1)
    valid_x = (in_x >= 0) & (in_x <= W - 1)
    top = np.floor(in_y)
    left = np.floor(in_x)
    y_lerp = (in_y - top).astype(np.float32)
    x_lerp = (in_x - left).astype(np.float32)
    ti = np.clip(top, 0, H - 1).astype(np.int32)
    bi = np.clip(top + 1, 0, H - 1).astype(np.int32)
    li = np.clip(left, 0, W - 1).astype(np.int32)
    ri = np.clip(left + 1, 0, W - 1).astype(np.int32)
    # patch start + effective in-patch lerp weights (boundary-exact: at the
    # high edge the reference's clipped pair collapses, which equals weight 1
    # on the previous pair; the low-edge collapse is invalid and masked)
    xs = np.minimum(li, W - 2).astype(np.int32)
    xw = np.where(li == ri, np.float32(1.0), x_lerp).astype(np.float32)
    ys = np.minimum(ti, H - 2).astype(np.int32)
    yw = np.where(ti == bi, np.float32(1.0), y_lerp).astype(np.float32)
    return ys, yw, xs, xw, valid_y, valid_x


def _build_core_inputs(boxes_k, img_off, n_real):
    """Gather indices + replicated bf16 corner weights for m_pad boxes.

    img_off: per-box patch-table offset (0 or NP4) selecting which of the
    two concatenated images this box samples from. (n_real is unused:
    SWDGE requires num_idxs_reg == count of non-negative descriptors, so
    dummy boxes keep real descriptors; their outputs are discarded.)
    """
    m_pad = boxes_k.shape[0]
    assert m_pad % CH == 0
    nch = m_pad // CH
    ys, yw, xs, xw, vy, vx = _grid_params(boxes_k)

    desc = (ys[:, :, None] * WP + xs[:, None, :]).reshape(m_pad, PX)
    desc = desc + img_off[:, None]
    vm = (vy[:, :, None] & vx[:, None, :]).astype(np.float32)
    yw2 = yw[:, :, None]
    xw2 = xw[:, None, :]
    w4 = np.empty((m_pad, 4, CROP, CROP), np.float32)
    w4[:, 0] = (1 - yw2) * (1 - xw2) * vm
    w4[:, 1] = (1 - yw2) * xw2 * vm
    w4[:, 2] = yw2 * (1 - xw2) * vm
    w4[:, 3] = yw2 * xw2 * vm
    w4 = w4.reshape(m_pad, 4, PX)

    idx_all = np.zeros((nch, NI), np.int16)
    w_all = np.zeros((nch, P, 4 * S * R), ml_dtypes.bfloat16)
    w3_all = np.zeros((nch, P, S), np.float32)
    for ch in range(nch):
        sl = slice(ch * CH, (ch + 1) * CH)
        d = desc[sl].reshape(-1)
        idx_all[ch, : d.size] = d
        wq = np.zeros((4, QPAD), np.float32)
        wq[:, : CH * PX] = w4[sl].transpose(1, 0, 2).reshape(4, -1)
        # pixel q = s*128 + p ; weight layout [p, ((k*S)+s)*R + r]
        wp = wq.reshape(4, S, P).transpose(2, 0, 1)           # [P, 4, S]
        w_all[ch] = np.broadcast_to(
            wp[:, :, :, None], (P, 4, S, R)).reshape(P, 4 * S * R)
        w3_all[ch] = wp[:, 3]
    # wrapped idx layout: [16, NI//16] idx k at (k%16, k//16), tiled to 128
    wrapped = idx_all.reshape(nch, NI // 16, 16).transpose(0, 2, 1)
    idx_wrapped = np.tile(wrapped, (1, NCORES, 1))  # [nch, 128, NI//16]
    return idx_wrapped, w_all, w3_all


def _build_program(nch):
    nc = bacc.Bacc("TRN2", target_bir_lowering=False, debug=False,
                   num_devices=NCORES, num_swdge_queues=4)
    img = nc.dram_tensor("img", [2 * NP4 * EL], BF16, kind="ExternalInput")
    idx = nc.dram_tensor("idx", [nch, P, NI // 16], I16, kind="ExternalInput")
    wts = nc.dram_tensor("wts", [nch, P, 4 * S * R], BF16,
                         kind="ExternalInput")
    wt3 = nc.dram_tensor("wt3", [nch, P, S], F32, kind="ExternalInput")
    out = nc.dram_tensor("out", [nch * QPAD * C], BF16, kind="ExternalOutput")

    gather_src = bass.AP(img, 0, [(EL, 2 * NP4), (1, EL)])

    with tile.TileContext(nc) as tc:
        with (
            tc.tile_pool(name="gat", bufs=3) as gat_pool,
            tc.tile_pool(name="osb", bufs=3) as out_pool,
            tc.tile_pool(name="meta", bufs=3) as meta_pool,
            tc.tile_pool(name="tmp", bufs=2) as tmp_pool,
        ):
            nc.gpsimd.load_library(library_config.mlp)
            for ch in range(nch):
                idx_t = meta_pool.tile([P, NI // 16], I16, tag="idx")
                nc.sync.dma_start(idx_t[:], idx[ch])
                w_t = meta_pool.tile([P, 4, S, R], BF16, tag="wts")
                nc.sync.dma_start(w_t[:], wts[ch])
                w3_t = meta_pool.tile([P, S], F32, tag="wt3")
                nc.sync.dma_start(w3_t[:], wt3[ch])

                g = gat_pool.tile([P, S, EL], BF16, tag="g")
                # SWDGE ring limit: 512-desc sub-gathers are safe; cycle the
                # 4 SWDGE queues so desc-gen never stalls on ring space.
                GU = 512
                for qn, j0 in enumerate(range(0, NI, GU)):
                    nj = min(GU, NI - j0)
                    nc.gpsimd.dma_gather(
                        g[:, j0 // P: (j0 + nj) // P, :], gather_src,
                        idx_t[:, j0 // 16: (j0 + nj) // 16], nj, nj,
                        EL, elem_step=EL, queue_num=qn % 4)

                o = out_pool.tile([P, S, C], BF16, tag="o")
                t0 = tmp_pool.tile([P, S, C], BF16, tag="t0")
                t1 = tmp_pool.tile([P, S, C], BF16, tag="t1")
                t0b = tmp_pool.tile([P, S, C], BF16, tag="t0")
                t1b = tmp_pool.tile([P, S, C], BF16, tag="t1")
                t2 = tmp_pool.tile([P, S, C], BF16, tag="t2")
                t2b = tmp_pool.tile([P, S, C], BF16, tag="t2")

                gf = g[:]

                def gk(k):
                    # corner k of the gathered payload as [P, S, CB, R]
                    return bass.AP(gf.tensor, gf.offset + k * C,
                                   [gf.ap[0], (EL, S), (R, CB), (1, R)])

                def vk(t):
                    # [P, S, C] tile viewed as [P, S, CB, R]
                    a = t[:]
                    return bass.AP(a.tensor, a.offset,
                                   [a.ap[0], (C, S), (R, CB), (1, R)])

                def wk(k):
                    return w_t[:, k].unsqueeze(2).to_broadcast([P, S, CB, R])

                # corner 3 product runs on the otherwise-idle Act engine as
                # 13 per-slot per-partition-scalar muls, freeing a DVE op
                for s in range(S):
                    nc.scalar.mul(t1b[:, s, :], g[:, s, 3 * C: 4 * C],
                                  w3_t[:, s: s + 1])
                nc.vector.tensor_tensor(vk(t0), gk(0), wk(0), MULT)
                nc.vector.tensor_tensor(vk(t1), gk(1), wk(1), MULT)
                nc.vector.tensor_tensor(t2[:], t0[:], t1[:], ADD)
                nc.vector.tensor_tensor(vk(t0b), gk(2), wk(2), MULT)
                nc.vector.tensor_tensor(t2b[:], t0b[:], t1b[:], ADD)
                nc.vector.tensor_tensor(o[:], t2[:], t2b[:], ADD)

                out_ap = bass.AP(out, ch * QPAD * C,
                                 [(C, P), (P * C, S), (1, C)])
                nc.scalar.dma_start(out_ap, o[:])

    nc.compile()
    return nc


def _build_img4(image_t):
    """image_t: [B, H, W, C] f32 channel-last -> [B, NP4*EL] bf16."""
    B = image_t.shape[0]
    img4 = np.empty((B, HP, WP, 4, C), ml_dtypes.bfloat16)
    img4[:, :, :, 0] = image_t[:, :HP, :WP]
    img4[:, :, :, 1] = image_t[:, :HP, 1:]
    img4[:, :, :, 2] = image_t[:, 1:, :WP]
    img4[:, :, :, 3] = image_t[:, 1:, 1:]
    return img4.reshape(B, NP4 * EL)


def kernel(image, boxes, box_ind):
    image = np.asarray(image, dtype=np.float32)
    boxes = np.asarray(boxes, dtype=np.float32)
    box_ind = np.asarray(box_ind)
    n_boxes = boxes.shape[0]
    B = image.shape[0]

    # balanced contiguous windows over boxes sorted by image id, with each
    # window capped to touch at most 2 images (int16 descriptor range only
    # covers a 2-image patch table); fall back to per-image grouping if the
    # greedy cut can't satisfy that
    order = np.argsort(box_ind, kind="stable")
    sorted_ind = box_ind[order]
    # cum[j] = first position of image >= j in the sorted order
    cum = np.searchsorted(sorted_ind, np.arange(B + 1))
    bounds = [0]
    for k in range(1, NCORES):
        tgt = round(k * n_boxes / NCORES)
        lo = bounds[k - 1]
        img_lo = int(np.searchsorted(cum, lo, side="right")) - 1  # img at lo
        cap = int(cum[min(img_lo + 2, B)])  # end of image img_lo+1
        bounds.append(max(lo, min(tgt, cap)))
    bounds.append(n_boxes)
    wins = [order[bounds[k]: bounds[k + 1]] for k in range(NCORES)]
    if not all(len(np.unique(box_ind[w])) <= 2 for w in wins):
        wins = [np.where(box_ind == k % B)[0] for k in range(NCORES)]

    m_max = max(max(len(w) for w in wins), 1)
    m_pad = ((m_max + CH - 1) // CH) * CH
    nch = m_pad // CH
    dummy = np.array([[0.25, 0.25, 0.75, 0.75]], np.float32)

    image_t = np.ascontiguousarray(image.transpose(0, 2, 3, 1))  # [B,H,W,C]
    img4 = _build_img4(image_t)

    in_maps = []
    for k in range(NCORES):
        w = wins[k]
        u = np.unique(box_ind[w]) if len(w) else np.array([0])
        a, b = int(u[0]), int(u[-1])
        bk = boxes[w]
        img_off = np.where(box_ind[w] == b, NP4, 0) if b != a else \
            np.zeros(len(w), np.int32)
        if bk.shape[0] < m_pad:
            pad = m_pad - bk.shape[0]
            bk = np.concatenate([bk, np.repeat(dummy, pad, 0)], axis=0)
            img_off = np.concatenate([img_off, np.zeros(pad, np.int32)])
        idx_w, w_all, w3_all = _build_core_inputs(
            bk, img_off.astype(np.int32), len(w))
        in_maps.append({
            "img": np.concatenate([img4[a], img4[b]]),
            "idx": idx_w,
            "wts": w_all,
            "wt3": w3_all,
        })

    key = nch
    if key not in _cache:
        _cache[key] = _build_program(nch)
    nc = _cache[key]

    res = bass_utils.run_bass_kernel_spmd(nc, in_maps,
                                          core_ids=list(range(NCORES)))
    global LAST_EXEC_NS
    LAST_EXEC_NS = res.exec_time_ns

    out = np.zeros((n_boxes, C, CROP, CROP), np.float32)
    for k in range(NCORES):
        ok = np.asarray(res.results[k]["out"]).reshape(nch, QPAD, C)
        ok = ok[:, : CH * PX, :].reshape(m_pad, PX, C)[: len(wins[k])]
        out[wins[k]] = ok.transpose(0, 2, 1).reshape(
            -1, C, CROP, CROP).astype(np.float32)
    return out
